# revision 29
# baseline (speedup 1.0000x reference)
"""DifferenceOfGaussiansFFT on 8 Trainium2 NeuronCores — v3.

Sharding: core c -> (batch b = c//4, quarter q = c%4).  Each core computes
dog planes [8q, 8q+8) for its batch plus one halo plane each side
(recompute, no collectives): 11 blur slots, 10 dogs, 8 pools per core.

Math per core (all layouts TRANSPOSED: partition = x = W, free = y = H):
    pass1: S1[y, x'] = sum_x T[x, y] A[x, x']     (stationary image blocks,
           moving banded A windows; per-slot window width 128+2R_t)
    pass2: U^T[x', y'] = sum_y S1[y, x'] A[y, y'] (stationary S1 blocks,
           SAME moving A windows — A is symmetric)
    W_t   = sigma_t*U_t - thn                (ACT eviction, scale+bias)
    dog_{t-1} = psB_t*(-sigma_{t-1}*2^-30) + W_{t-1}   (one STT from PSUM)
    lm'   = maxpool3d(dog')  (F via q/tri, H free-dim shifts, W via DRAM
            round trip for partition shifts)
    mask  = ((lm' max 0) is_equal dog')      (one fused STT, u8)
    host adds thn back and transposes [x,y] -> [y,x].

Matmuls are bf16-pair fp16 hi/lo, 3 terms (Ah*Bh + Al*Bh + Ah*Bl),
weights scaled 2^15 per pass (U comes out 2^30-scaled; compensated in the
ACT/STT scale tables).  PSUM start flag zeroes the whole bank, so only the
first matmul per bank sets start=True.
"""

import math

import numpy as np

_IMG = 512
_B = 2
_F = 33
_R = 51
_TH = 0.001
_NCORES = 8
_NS1 = 11   # blur slots per core
_ND = 10    # dog slots per core

# per-slot radius: slot t's worst filter over cores is q=3's f = min(23+t, 32)
def _slot_radii(sigmas):
    rads = [int(5.0 * float(s) + 0.5) for s in sigmas]
    return [rads[min(23 + t, _F - 1)] for t in range(_NS1)]


def _windows(Rt):
    # (jlo, width) per kt; coverage of band [128kt-R, 128kt+128+R) clipped
    return [
        (0, 128 + Rt),
        (128 - Rt, 128 + 2 * Rt),
        (256 - Rt, 128 + 2 * Rt),
        (384 - Rt, 128 + Rt),
    ]


_cache = {}


def _thn():
    return float(np.nextafter(np.float32(_TH), np.float32(np.inf)))


def _build_host_data(kernels, sigmas):
    fp16 = np.float16
    kernels = np.asarray(kernels, dtype=np.float32)
    sigmas = np.asarray(sigmas, dtype=np.float32)
    F = kernels.shape[0]
    assert F == _F

    # exact 1D taps: kernel = outer(t, t) with t = row / sqrt(center)
    A32 = np.zeros((F, _IMG, _IMG), dtype=np.float32)
    idx = np.arange(_IMG)
    for f in range(F):
        k2 = kernels[f].astype(np.float64)
        taps = k2[_R, : 2 * _R + 1] / math.sqrt(k2[_R, _R])
        A = np.zeros((_IMG, _IMG), dtype=np.float64)
        for d in range(-_R, _R + 1):
            v = taps[_R + d]
            src = idx[max(0, -d): _IMG - max(0, d)]
            A[src, src + d] = v
        A32[f] = A.astype(np.float32)

    radii = _slot_radii(sigmas)

    def pair(x):
        h = x.astype(fp16)
        l = (x - h.astype(np.float32)).astype(fp16)
        return np.ascontiguousarray(h), np.ascontiguousarray(l)

    # per-quarter banded window tensors [NS1, 128, 1024], fp16 pair, x 2^15
    aw_by_q = {}
    for qq in range(4):
        J0 = 8 * qq - 1
        gs = [min(max(J0 + t, 0), _F - 1) for t in range(_NS1)]
        aw = np.zeros((_NS1, 128, 1024), dtype=np.float32)
        for t in range(_NS1):
            f = gs[t]
            for kt, (jlo, w) in enumerate(_windows(radii[t])):
                rows = A32[f][128 * kt: 128 * kt + 128]
                aw[t, :, 256 * kt: 256 * kt + w] = rows[:, jlo: jlo + w]
        aw_by_q[qq] = pair(aw * np.float32(2.0 ** 15))

    return aw_by_q, sigmas, radii, fp16


def _build_program(radii):
    import concourse.bass as bass  # noqa: F401
    import concourse.mybir as mybir
    import concourse.tile as tile
    from concourse import bacc

    fp32 = mybir.dt.float32
    fp16 = mybir.dt.float16
    u8 = mybir.dt.uint8
    Alu = mybir.AluOpType
    Act = mybir.ActivationFunctionType

    nc = bacc.Bacc("TRN2", target_bir_lowering=False)

    Th_d = nc.dram_tensor("timgh", [_IMG, _IMG], fp16, kind="ExternalInput")
    Tl_d = nc.dram_tensor("timgl", [_IMG, _IMG], fp16, kind="ExternalInput")
    awh_d = nc.dram_tensor("awh", [_NS1, 128, 1024], fp16, kind="ExternalInput")
    awl_d = nc.dram_tensor("awl", [_NS1, 128, 1024], fp16, kind="ExternalInput")
    # scale/bias tables: [128, NS1] each
    wsc_d = nc.dram_tensor("wsc", [128, _NS1], fp32, kind="ExternalInput")
    wbi_d = nc.dram_tensor("wbi", [128, _NS1], fp32, kind="ExternalInput")
    dsc_d = nc.dram_tensor("dsc", [128, _NS1], fp32, kind="ExternalInput")
    lm_d = nc.dram_tensor("lm", [8, _IMG, _IMG], fp32, kind="ExternalOutput")
    mk_d = nc.dram_tensor("mask", [8, _IMG, _IMG], u8, kind="ExternalOutput")

    with tile.TileContext(nc) as tc:
        with (
            tc.tile_pool(name="const", bufs=1) as constp,
            tc.tile_pool(name="aw", bufs=2) as awp,
            tc.tile_pool(name="s1", bufs=2) as s1p,
            tc.tile_pool(name="s1f", bufs=1) as s1fp,
            tc.tile_pool(name="wv", bufs=2) as wvp,
            tc.tile_pool(name="dv", bufs=2) as dvp,
            tc.tile_pool(name="dog", bufs=3) as dogp,
            tc.tile_pool(name="q", bufs=2) as qp,
            tc.tile_pool(name="tri", bufs=1) as trip,
            tc.tile_pool(name="m1", bufs=1) as m1p,
            tc.tile_pool(name="mh", bufs=2) as mhp,
            tc.tile_pool(name="sh", bufs=4) as shp,
            tc.tile_pool(name="hx", bufs=1) as hxp,
            tc.tile_pool(name="lmp", bufs=1) as lmp,
            tc.tile_pool(name="msk", bufs=1) as mskp,
            tc.tile_pool(name="hs", bufs=3, space="DRAM") as hsp,
            tc.tile_pool(name="psA", bufs=4, space="PSUM") as psAp,
            tc.tile_pool(name="psB", bufs=1, space="PSUM") as psBp,
        ):
            Th_sb = constp.tile([128, 4, _IMG], fp16, tag="th")
            nc.sync.dma_start(Th_sb[:], Th_d.rearrange("(t p) y -> p t y", p=128))
            Tl_sb = constp.tile([128, 4, _IMG], fp16, tag="tl")
            nc.sync.dma_start(Tl_sb[:], Tl_d.rearrange("(t p) y -> p t y", p=128))
            wsc_sb = constp.tile([128, _NS1], fp32, tag="wsc")
            nc.sync.dma_start(wsc_sb[:], wsc_d[:])
            wbi_sb = constp.tile([128, _NS1], fp32, tag="wbi")
            nc.sync.dma_start(wbi_sb[:], wbi_d[:])
            dsc_sb = constp.tile([128, _NS1], fp32, tag="dsc")
            nc.sync.dma_start(dsc_sb[:], dsc_d[:])

            AW = {}
            S1 = {}
            WV = {}   # W_t = sigma_t * U_t - thn
            PSB = {}
            DOG = {}
            Q = {}

            def load_aw(t):
                awh = awp.tile([128, 1024], fp16, tag="awh")
                nc.sync.dma_start(awh[:], awh_d[t])
                awl = awp.tile([128, 1024], fp16, tag="awl")
                nc.sync.dma_start(awl[:], awl_d[t])
                AW[t] = (awh, awl)

            def mt_matmuls(t, stat_h, stat_l, out_ap_fn):
                """12 matmuls per mt: 4 kt x 3 hi/lo terms into psum."""
                awh, awl = AW[t]
                wins = _windows(radii[t])
                for mt in range(4):
                    nmm = 0
                    for kt in (0, 3, 1, 2):
                        jlo, w = wins[kt]
                        terms = ((stat_h, awh), (stat_l, awh), (stat_h, awl))
                        for sb, aw in terms:
                            nc.tensor.matmul(
                                out_ap_fn(mt, jlo, w),
                                sb[:, kt, 128 * mt: 128 * mt + 128],
                                aw[:, 256 * kt: 256 * kt + w],
                                start=(nmm == 0),
                                stop=(nmm == 11),
                            )
                            nmm += 1

            def slot(t):
                # pass1: 4 separate 1-bank psum tiles for per-mt eviction
                # pipelining (PE can roll into pass2 while evictions drain)
                psA = [psAp.tile([128, _IMG], fp32, tag="ps", name=f"psA{t}_{m}")
                       for m in range(4)]
                mt_matmuls(t, Th_sb, Tl_sb,
                           lambda mt, jlo, w: psA[mt][:, jlo: jlo + w])
                # dual eviction keeps the lo-residual OFF the vector engine:
                # scalar evicts both fp16-hi and fp32 copies; gpsimd (which
                # cannot read PSUM) then forms lo = f32 - hi from SBUF.
                s1h = s1p.tile([128, 4, _IMG], fp16, tag="s1h")
                s1l = s1p.tile([128, 4, _IMG], fp16, tag="s1l")
                s1f = s1fp.tile([128, 4, _IMG], fp32, tag="s1f")
                for mt in range(4):
                    nc.scalar.activation(s1h[:, mt, :], psA[mt][:], Act.Copy)
                    nc.scalar.activation(s1f[:, mt, :], psA[mt][:], Act.Copy)
                    nc.gpsimd.tensor_tensor(
                        s1l[:, mt, :], s1f[:, mt, :], s1h[:, mt, :],
                        Alu.subtract)
                # pass2: one 4-bank psum tile (consumers are not PE-critical)
                psB = psBp.tile([128, 4, _IMG], fp32, tag="psb")
                mt_matmuls(t, s1h, s1l,
                           lambda mt, jlo, w: psB[:, mt, jlo: jlo + w])
                S1[t] = (s1h, s1l)
                if t - 1 in S1:
                    del S1[t - 1]
                # W_t eviction (scalar): W = psB*wsc[t] + wbi[t]
                wv = wvp.tile([128, 4, _IMG], fp32, tag="wv")
                nc.scalar.activation(
                    wv[:], psB[:], Act.Identity,
                    scale=wsc_sb[:, t: t + 1], bias=wbi_sb[:, t: t + 1])
                WV[t] = wv
                # D_t = psB_t * dsc[t] (scalar ACT), then
                # dog_{t-1} = D_t + W_{t-1} on gpsimd (the only engine with
                # spare capacity; it cannot read PSUM or do max, but add is
                # in its supported op set)
                if t >= 1:
                    dsb = dvp.tile([128, 4, _IMG], fp32, tag="dv")
                    nc.scalar.activation(
                        dsb[:], psB[:], Act.Identity,
                        scale=dsc_sb[:, t: t + 1])
                    d = dogp.tile([128, 4, _IMG], fp32, tag="dog")
                    nc.gpsimd.tensor_tensor(
                        d[:], dsb[:], WV[t - 1][:], Alu.add)
                    DOG[t - 1] = d
                    del WV[t - 1]

            def make_q(x):
                # q_x = max(dog_x, dog_{x+1})   (max only exists on DVE)
                qt = qp.tile([128, 4, _IMG], fp32, tag="q")
                nc.vector.tensor_tensor(qt[:], DOG[x][:], DOG[x + 1][:], Alu.max)
                Q[x] = qt

            SH = {}

            def pool_front(o):
                # output plane o (0..7): tri over dogs o, o+1, o+2; then the
                # free-dim (H) 3-max and the W round-trip DMAs.  The round
                # trip completes during the NEXT slot; pool_back picks it up.
                tri = trip.tile([128, 4, _IMG], fp32, tag="tri")
                nc.vector.tensor_tensor(
                    tri[:], Q[o][:], DOG[o + 2][:], Alu.max)
                if o - 1 in Q:
                    del Q[o - 1]

                # H (y, free dim) 3-max; edge columns via scalar copies
                m1 = m1p.tile([128, 4, _IMG], fp32, tag="m1")
                nc.vector.tensor_tensor(
                    m1[:, :, 0:511], tri[:, :, 0:511], tri[:, :, 1:512],
                    Alu.max)
                nc.scalar.activation(
                    m1[:, :, 511:512], tri[:, :, 511:512], Act.Copy)
                mh = mhp.tile([128, 4, _IMG], fp32, tag="mh")
                nc.vector.tensor_tensor(
                    mh[:, :, 1:512], m1[:, :, 0:511], m1[:, :, 1:512], Alu.max)
                nc.scalar.activation(mh[:, :, 0:1], m1[:, :, 0:1], Act.Copy)

                # W (x = partition dim) shifts via DRAM round trip, edge clamp
                hs = hsp.tile([514, _IMG], fp32, tag="hs")
                nc.sync.dma_start(
                    hs[1:513].rearrange("(m p) y -> p m y", p=128), mh[:])
                nc.scalar.dma_start(hs[0:1], mh[0:1, 0:1, :])
                nc.scalar.dma_start(hs[513:514], mh[127:128, 3:4, :])
                shA = shp.tile([128, 4, _IMG], fp32, tag="sh")
                nc.sync.dma_start(
                    shA[:], hs[0:512].rearrange("(m p) y -> p m y", p=128))
                shB = shp.tile([128, 4, _IMG], fp32, tag="sh")
                nc.sync.dma_start(
                    shB[:], hs[2:514].rearrange("(m p) y -> p m y", p=128))
                SH[o] = (mh, shA, shB)

            def pool_back(o):
                mh, shA, shB = SH.pop(o)
                hA = hxp.tile([128, 4, _IMG], fp32, tag="hx")
                nc.vector.tensor_tensor(hA[:], mh[:], shA[:], Alu.max)
                lmT = lmp.tile([128, 4, _IMG], fp32, tag="lm")
                nc.vector.tensor_tensor(lmT[:], hA[:], shB[:], Alu.max)
                nc.sync.dma_start(
                    lm_d[o].rearrange("(m p) y -> p m y", p=128), lmT[:])

                # mask = ((lm' max 0) == dog'_{o+1})   (one fused STT)
                mk = mskp.tile([128, 4, _IMG], u8, tag="msk")
                nc.vector.scalar_tensor_tensor(
                    mk[:], lmT[:], 0.0, DOG[o + 1][:], Alu.max, Alu.is_equal)
                nc.sync.dma_start(
                    mk_d[o].rearrange("(m p) y -> p m y", p=128), mk[:])
                if o in DOG:
                    del DOG[o]

            load_aw(0)
            for t in range(_NS1):
                if t + 1 < _NS1:
                    load_aw(t + 1)
                slot(t)
                if 2 <= t <= 9:
                    make_q(t - 2)
                if t >= 3:
                    pool_front(t - 3)
                if t >= 4:
                    pool_back(t - 4)
            pool_back(7)

    nc.compile()
    return nc


def kernel(input, kernels, sigmas):
    from concourse.bass_utils import run_bass_kernel_spmd

    input = np.asarray(input, dtype=np.float32)
    aw_by_q, sig, radii, bf16 = _cache.setdefault(
        "host", _build_host_data(kernels, sigmas))

    if "prog" not in _cache:
        _cache["prog"] = _build_program(radii)
    nc = _cache["prog"]

    thn = _thn()
    in_maps = []
    for c in range(_NCORES):
        b, qq = c // 4, c % 4
        J0 = 8 * qq - 1

        T = np.ascontiguousarray(np.transpose(input[b]))  # [x, y]
        Th = T.astype(bf16)
        Tl = (T - Th.astype(np.float32)).astype(bf16)

        wsc = np.zeros((128, _NS1), dtype=np.float32)
        wbi = np.zeros((128, _NS1), dtype=np.float32)
        dsc = np.zeros((128, _NS1), dtype=np.float32)
        for t in range(_NS1):
            j = J0 + t
            if 0 <= j < _F - 1:
                wsc[:, t] = sig[j] * np.float32(2.0 ** -30)
                wbi[:, t] = -thn
            else:
                wsc[:, t] = 0.0
                wbi[:, t] = -1e38
            jd = J0 + t - 1
            if 0 <= jd < _F - 1:
                dsc[:, t] = -sig[jd] * np.float32(2.0 ** -30)
            else:
                dsc[:, t] = 0.0
        awh, awl = aw_by_q[qq]
        in_maps.append({
            "timgh": Th, "timgl": Tl,
            "awh": awh, "awl": awl,
            "wsc": wsc, "wbi": wbi, "dsc": dsc,
        })

    res = run_bass_kernel_spmd(
        nc, in_maps, core_ids=list(range(_NCORES)),
        trace=_cache.get("trace", False),
        tmpdir=_cache.get("tmpdir"),
    )
    _cache["last_res"] = res

    lm_full = np.empty((_B, _F - 1, _IMG, _IMG), dtype=np.float32)
    mk_full = np.empty((_B, _F - 1, _IMG, _IMG), dtype=bool)
    for c in range(_NCORES):
        b, qq = c // 4, c % 4
        lm_c = res.results[c]["lm"]      # [8, x, y]
        mk_c = res.results[c]["mask"]
        lm_full[b, 8 * qq: 8 * qq + 8] = (
            np.transpose(lm_c, (0, 2, 1)) + np.float32(thn))
        mk_full[b, 8 * qq: 8 * qq + 8] = np.transpose(mk_c, (0, 2, 1)) != 0
    return mk_full, lm_full


# revision 31
# speedup vs baseline: 1.1346x; 1.1346x over previous
"""DifferenceOfGaussiansFFT on 8 Trainium2 NeuronCores — v3.

Sharding: core c -> (batch b = c//4, quarter q = c%4).  Each core computes
dog planes [8q, 8q+8) for its batch plus one halo plane each side
(recompute, no collectives): 11 blur slots, 10 dogs, 8 pools per core.

Math per core (all layouts TRANSPOSED: partition = x = W, free = y = H):
    pass1: S1[y, x'] = sum_x T[x, y] A[x, x']     (stationary image blocks,
           moving banded A windows; per-slot window width 128+2R_t)
    pass2: U^T[x', y'] = sum_y S1[y, x'] A[y, y'] (stationary S1 blocks,
           SAME moving A windows — A is symmetric)
    W_t   = sigma_t*U_t - thn                (ACT eviction, scale+bias)
    dog_{t-1} = psB_t*(-sigma_{t-1}*2^-30) + W_{t-1}   (one STT from PSUM)
    lm'   = maxpool3d(dog')  (F via q/tri, H free-dim shifts, W via DRAM
            round trip for partition shifts)
    mask  = ((lm' max 0) is_equal dog')      (one fused STT, u8)
    host adds thn back and transposes [x,y] -> [y,x].

Matmuls are bf16-pair fp16 hi/lo, 3 terms (Ah*Bh + Al*Bh + Ah*Bl),
weights scaled 2^15 per pass (U comes out 2^30-scaled; compensated in the
ACT/STT scale tables).  PSUM start flag zeroes the whole bank, so only the
first matmul per bank sets start=True.
"""

import math

import numpy as np

_IMG = 512
_B = 2
_F = 33
_R = 51
_TH = 0.001
_NCORES = 8
_NS1 = 11   # blur slots per core
_ND = 10    # dog slots per core

# per-slot radius: slot t's worst filter over cores is q=3's f = min(23+t, 32)
def _slot_radii(sigmas):
    rads = [int(5.0 * float(s) + 0.5) for s in sigmas]
    return [rads[min(23 + t, _F - 1)] for t in range(_NS1)]


def _windows(Rt):
    # (jlo, width) per kt; coverage of band [128kt-R, 128kt+128+R) clipped
    return [
        (0, 128 + Rt),
        (128 - Rt, 128 + 2 * Rt),
        (256 - Rt, 128 + 2 * Rt),
        (384 - Rt, 128 + Rt),
    ]


_cache = {}


def _thn():
    return float(np.nextafter(np.float32(_TH), np.float32(np.inf)))


def _build_host_data(kernels, sigmas):
    fp16 = np.float16
    kernels = np.asarray(kernels, dtype=np.float32)
    sigmas = np.asarray(sigmas, dtype=np.float32)
    F = kernels.shape[0]
    assert F == _F

    # exact 1D taps: kernel = outer(t, t) with t = row / sqrt(center)
    A32 = np.zeros((F, _IMG, _IMG), dtype=np.float32)
    idx = np.arange(_IMG)
    for f in range(F):
        k2 = kernels[f].astype(np.float64)
        taps = k2[_R, : 2 * _R + 1] / math.sqrt(k2[_R, _R])
        A = np.zeros((_IMG, _IMG), dtype=np.float64)
        for d in range(-_R, _R + 1):
            v = taps[_R + d]
            src = idx[max(0, -d): _IMG - max(0, d)]
            A[src, src + d] = v
        A32[f] = A.astype(np.float32)

    radii = _slot_radii(sigmas)

    def pair(x):
        h = x.astype(fp16)
        l = (x - h.astype(np.float32)).astype(fp16)
        return np.ascontiguousarray(h), np.ascontiguousarray(l)

    # per-quarter banded window tensors [NS1, 128, 1024], fp16 pair, x 2^15
    aw_by_q = {}
    for qq in range(4):
        J0 = 8 * qq - 1
        gs = [min(max(J0 + t, 0), _F - 1) for t in range(_NS1)]
        aw = np.zeros((_NS1, 128, 1024), dtype=np.float32)
        for t in range(_NS1):
            f = gs[t]
            for kt, (jlo, w) in enumerate(_windows(radii[t])):
                rows = A32[f][128 * kt: 128 * kt + 128]
                aw[t, :, 256 * kt: 256 * kt + w] = rows[:, jlo: jlo + w]
        aw_by_q[qq] = pair(aw * np.float32(2.0 ** 15))

    return aw_by_q, sigmas, radii, fp16


def _build_program(radii):
    import concourse.bass as bass  # noqa: F401
    import concourse.mybir as mybir
    import concourse.tile as tile
    from concourse import bacc

    fp32 = mybir.dt.float32
    fp16 = mybir.dt.float16
    u8 = mybir.dt.uint8
    Alu = mybir.AluOpType
    Act = mybir.ActivationFunctionType

    nc = bacc.Bacc("TRN2", target_bir_lowering=False)

    Th_d = nc.dram_tensor("timgh", [_IMG, _IMG], fp16, kind="ExternalInput")
    Tl_d = nc.dram_tensor("timgl", [_IMG, _IMG], fp16, kind="ExternalInput")
    awh_d = nc.dram_tensor("awh", [_NS1, 128, 1024], fp16, kind="ExternalInput")
    awl_d = nc.dram_tensor("awl", [_NS1, 128, 1024], fp16, kind="ExternalInput")
    # scale/bias tables: [128, NS1] each
    wsc_d = nc.dram_tensor("wsc", [128, _NS1], fp32, kind="ExternalInput")
    wbi_d = nc.dram_tensor("wbi", [128, _NS1], fp32, kind="ExternalInput")
    dsc_d = nc.dram_tensor("dsc", [128, _NS1], fp32, kind="ExternalInput")
    lm_d = nc.dram_tensor("lm", [8, _IMG, _IMG], fp32, kind="ExternalOutput")
    mk_d = nc.dram_tensor("mask", [8, _IMG, _IMG], u8, kind="ExternalOutput")

    with tile.TileContext(nc) as tc:
        with (
            tc.tile_pool(name="const", bufs=1) as constp,
            tc.tile_pool(name="aw", bufs=2) as awp,
            tc.tile_pool(name="s1", bufs=2) as s1p,
            tc.tile_pool(name="wv", bufs=2) as wvp,
            tc.tile_pool(name="dv", bufs=2) as dvp,
            tc.tile_pool(name="dog", bufs=3) as dogp,
            tc.tile_pool(name="q", bufs=2) as qp,
            tc.tile_pool(name="tri", bufs=1) as trip,
            tc.tile_pool(name="m1", bufs=1) as m1p,
            tc.tile_pool(name="mh", bufs=2) as mhp,
            tc.tile_pool(name="sh", bufs=4) as shp,
            tc.tile_pool(name="hx", bufs=1) as hxp,
            tc.tile_pool(name="lmp", bufs=1) as lmp,
            tc.tile_pool(name="msk", bufs=1) as mskp,
            tc.tile_pool(name="hs", bufs=3, space="DRAM") as hsp,
            tc.tile_pool(name="psA", bufs=4, space="PSUM") as psAp,
            tc.tile_pool(name="psB", bufs=1, space="PSUM") as psBp,
        ):
            Th_sb = constp.tile([128, 4, _IMG], fp16, tag="th")
            nc.sync.dma_start(Th_sb[:], Th_d.rearrange("(t p) y -> p t y", p=128))
            Tl_sb = constp.tile([128, 4, _IMG], fp16, tag="tl")
            nc.sync.dma_start(Tl_sb[:], Tl_d.rearrange("(t p) y -> p t y", p=128))
            wsc_sb = constp.tile([128, _NS1], fp32, tag="wsc")
            nc.sync.dma_start(wsc_sb[:], wsc_d[:])
            wbi_sb = constp.tile([128, _NS1], fp32, tag="wbi")
            nc.sync.dma_start(wbi_sb[:], wbi_d[:])
            dsc_sb = constp.tile([128, _NS1], fp32, tag="dsc")
            nc.sync.dma_start(dsc_sb[:], dsc_d[:])

            AW = {}
            S1 = {}
            WV = {}   # W_t = sigma_t * U_t - thn
            PSB = {}
            DOG = {}
            Q = {}

            def load_aw(t):
                awh = awp.tile([128, 1024], fp16, tag="awh")
                nc.sync.dma_start(awh[:], awh_d[t])
                awl = awp.tile([128, 1024], fp16, tag="awl")
                nc.sync.dma_start(awl[:], awl_d[t])
                AW[t] = (awh, awl)

            def mt_matmuls(t, stat_h, stat_l, out_ap_fn):
                """12 matmuls per mt: 4 kt x 3 hi/lo terms into psum."""
                awh, awl = AW[t]
                wins = _windows(radii[t])
                for mt in range(4):
                    nmm = 0
                    for kt in (0, 3, 1, 2):
                        jlo, w = wins[kt]
                        terms = ((stat_h, awh), (stat_l, awh), (stat_h, awl))
                        for sb, aw in terms:
                            nc.tensor.matmul(
                                out_ap_fn(mt, jlo, w),
                                sb[:, kt, 128 * mt: 128 * mt + 128],
                                aw[:, 256 * kt: 256 * kt + w],
                                start=(nmm == 0),
                                stop=(nmm == 11),
                            )
                            nmm += 1

            def slot(t):
                # pass1: 4 separate 1-bank psum tiles for per-mt eviction
                # pipelining (PE can roll into pass2 while evictions drain)
                psA = [psAp.tile([128, _IMG], fp32, tag="ps", name=f"psA{t}_{m}")
                       for m in range(4)]
                mt_matmuls(t, Th_sb, Tl_sb,
                           lambda mt, jlo, w: psA[mt][:, jlo: jlo + w])
                s1h = s1p.tile([128, 4, _IMG], fp16, tag="s1h")
                s1l = s1p.tile([128, 4, _IMG], fp16, tag="s1l")
                for mt in range(4):
                    nc.scalar.activation(s1h[:, mt, :], psA[mt][:], Act.Copy)
                    nc.vector.tensor_tensor(
                        s1l[:, mt, :], psA[mt][:], s1h[:, mt, :], Alu.subtract)
                # pass2: one 4-bank psum tile (consumers are not PE-critical)
                psB = psBp.tile([128, 4, _IMG], fp32, tag="psb")
                mt_matmuls(t, s1h, s1l,
                           lambda mt, jlo, w: psB[:, mt, jlo: jlo + w])
                S1[t] = (s1h, s1l)
                if t - 1 in S1:
                    del S1[t - 1]
                # W_t eviction (scalar): W = psB*wsc[t] + wbi[t]
                wv = wvp.tile([128, 4, _IMG], fp32, tag="wv")
                nc.scalar.activation(
                    wv[:], psB[:], Act.Identity,
                    scale=wsc_sb[:, t: t + 1], bias=wbi_sb[:, t: t + 1])
                WV[t] = wv
                # D_t = psB_t * dsc[t] (scalar ACT), then
                # dog_{t-1} = D_t + W_{t-1} on gpsimd (the only engine with
                # spare capacity; it cannot read PSUM or do max, but add is
                # in its supported op set)
                if t >= 1:
                    dsb = dvp.tile([128, 4, _IMG], fp32, tag="dv")
                    nc.scalar.activation(
                        dsb[:], psB[:], Act.Identity,
                        scale=dsc_sb[:, t: t + 1])
                    d = dogp.tile([128, 4, _IMG], fp32, tag="dog")
                    nc.gpsimd.tensor_tensor(
                        d[:], dsb[:], WV[t - 1][:], Alu.add)
                    DOG[t - 1] = d
                    del WV[t - 1]

            def make_q(x):
                # q_x = max(dog_x, dog_{x+1})   (max only exists on DVE)
                qt = qp.tile([128, 4, _IMG], fp32, tag="q")
                nc.vector.tensor_tensor(qt[:], DOG[x][:], DOG[x + 1][:], Alu.max)
                Q[x] = qt

            SH = {}

            def pool_front(o):
                # output plane o (0..7): tri over dogs o, o+1, o+2; then the
                # free-dim (H) 3-max and the W round-trip DMAs.  The round
                # trip completes during the NEXT slot; pool_back picks it up.
                tri = trip.tile([128, 4, _IMG], fp32, tag="tri")
                nc.vector.tensor_tensor(
                    tri[:], Q[o][:], DOG[o + 2][:], Alu.max)
                if o - 1 in Q:
                    del Q[o - 1]

                # H (y, free dim) 3-max; edge columns via scalar copies
                m1 = m1p.tile([128, 4, _IMG], fp32, tag="m1")
                nc.vector.tensor_tensor(
                    m1[:, :, 0:511], tri[:, :, 0:511], tri[:, :, 1:512],
                    Alu.max)
                nc.scalar.activation(
                    m1[:, :, 511:512], tri[:, :, 511:512], Act.Copy)
                mh = mhp.tile([128, 4, _IMG], fp32, tag="mh")
                nc.vector.tensor_tensor(
                    mh[:, :, 1:512], m1[:, :, 0:511], m1[:, :, 1:512], Alu.max)
                nc.scalar.activation(mh[:, :, 0:1], m1[:, :, 0:1], Act.Copy)

                # W (x = partition dim) shifts via DRAM round trip, edge clamp
                hs = hsp.tile([514, _IMG], fp32, tag="hs")
                nc.sync.dma_start(
                    hs[1:513].rearrange("(m p) y -> p m y", p=128), mh[:])
                nc.scalar.dma_start(hs[0:1], mh[0:1, 0:1, :])
                nc.scalar.dma_start(hs[513:514], mh[127:128, 3:4, :])
                shA = shp.tile([128, 4, _IMG], fp32, tag="sh")
                nc.sync.dma_start(
                    shA[:], hs[0:512].rearrange("(m p) y -> p m y", p=128))
                shB = shp.tile([128, 4, _IMG], fp32, tag="sh")
                nc.sync.dma_start(
                    shB[:], hs[2:514].rearrange("(m p) y -> p m y", p=128))
                SH[o] = (mh, shA, shB)

            def pool_back(o):
                mh, shA, shB = SH.pop(o)
                hA = hxp.tile([128, 4, _IMG], fp32, tag="hx")
                nc.vector.tensor_tensor(hA[:], mh[:], shA[:], Alu.max)
                lmT = lmp.tile([128, 4, _IMG], fp32, tag="lm")
                nc.vector.tensor_tensor(lmT[:], hA[:], shB[:], Alu.max)
                nc.sync.dma_start(
                    lm_d[o].rearrange("(m p) y -> p m y", p=128), lmT[:])

                # mask = ((lm' max 0) == dog'_{o+1})   (one fused STT)
                mk = mskp.tile([128, 4, _IMG], u8, tag="msk")
                nc.vector.scalar_tensor_tensor(
                    mk[:], lmT[:], 0.0, DOG[o + 1][:], Alu.max, Alu.is_equal)
                nc.sync.dma_start(
                    mk_d[o].rearrange("(m p) y -> p m y", p=128), mk[:])
                if o in DOG:
                    del DOG[o]

            load_aw(0)
            for t in range(_NS1):
                if t + 1 < _NS1:
                    load_aw(t + 1)
                slot(t)
                if 2 <= t <= 9:
                    make_q(t - 2)
                if t >= 3:
                    pool_front(t - 3)
                if t >= 4:
                    pool_back(t - 4)
            pool_back(7)

    nc.compile()
    return nc


def kernel(input, kernels, sigmas):
    from concourse.bass_utils import run_bass_kernel_spmd

    input = np.asarray(input, dtype=np.float32)
    aw_by_q, sig, radii, bf16 = _cache.setdefault(
        "host", _build_host_data(kernels, sigmas))

    if "prog" not in _cache:
        _cache["prog"] = _build_program(radii)
    nc = _cache["prog"]

    thn = _thn()
    in_maps = []
    for c in range(_NCORES):
        b, qq = c // 4, c % 4
        J0 = 8 * qq - 1

        T = np.ascontiguousarray(np.transpose(input[b]))  # [x, y]
        Th = T.astype(bf16)
        Tl = (T - Th.astype(np.float32)).astype(bf16)

        wsc = np.zeros((128, _NS1), dtype=np.float32)
        wbi = np.zeros((128, _NS1), dtype=np.float32)
        dsc = np.zeros((128, _NS1), dtype=np.float32)
        for t in range(_NS1):
            j = J0 + t
            if 0 <= j < _F - 1:
                wsc[:, t] = sig[j] * np.float32(2.0 ** -30)
                wbi[:, t] = -thn
            else:
                wsc[:, t] = 0.0
                wbi[:, t] = -1e38
            jd = J0 + t - 1
            if 0 <= jd < _F - 1:
                dsc[:, t] = -sig[jd] * np.float32(2.0 ** -30)
            else:
                dsc[:, t] = 0.0
        awh, awl = aw_by_q[qq]
        in_maps.append({
            "timgh": Th, "timgl": Tl,
            "awh": awh, "awl": awl,
            "wsc": wsc, "wbi": wbi, "dsc": dsc,
        })

    res = run_bass_kernel_spmd(
        nc, in_maps, core_ids=list(range(_NCORES)),
        trace=_cache.get("trace", False),
        tmpdir=_cache.get("tmpdir"),
    )
    _cache["last_res"] = res

    lm_full = np.empty((_B, _F - 1, _IMG, _IMG), dtype=np.float32)
    mk_full = np.empty((_B, _F - 1, _IMG, _IMG), dtype=bool)
    for c in range(_NCORES):
        b, qq = c // 4, c % 4
        lm_c = res.results[c]["lm"]      # [8, x, y]
        mk_c = res.results[c]["mask"]
        lm_full[b, 8 * qq: 8 * qq + 8] = (
            np.transpose(lm_c, (0, 2, 1)) + np.float32(thn))
        mk_full[b, 8 * qq: 8 * qq + 8] = np.transpose(mk_c, (0, 2, 1)) != 0
    return mk_full, lm_full


# revision 33
# speedup vs baseline: 1.1481x; 1.0119x over previous
"""DifferenceOfGaussiansFFT on 8 Trainium2 NeuronCores — v3.

Sharding: core c -> (batch b = c//4, quarter q = c%4).  Each core computes
dog planes [8q, 8q+8) for its batch plus one halo plane each side
(recompute, no collectives): 11 blur slots, 10 dogs, 8 pools per core.

Math per core (all layouts TRANSPOSED: partition = x = W, free = y = H):
    pass1: S1[y, x'] = sum_x T[x, y] A[x, x']     (stationary image blocks,
           moving banded A windows; per-slot window width 128+2R_t)
    pass2: U^T[x', y'] = sum_y S1[y, x'] A[y, y'] (stationary S1 blocks,
           SAME moving A windows — A is symmetric)
    W_t   = sigma_t*U_t - thn                (ACT eviction, scale+bias)
    dog_{t-1} = psB_t*(-sigma_{t-1}*2^-30) + W_{t-1}   (one STT from PSUM)
    lm'   = maxpool3d(dog')  (F via q/tri, H free-dim shifts, W via DRAM
            round trip for partition shifts)
    mask  = ((lm' max 0) is_equal dog')      (one fused STT, u8)
    host adds thn back and transposes [x,y] -> [y,x].

Matmuls are bf16-pair fp16 hi/lo, 3 terms (Ah*Bh + Al*Bh + Ah*Bl),
weights scaled 2^15 per pass (U comes out 2^30-scaled; compensated in the
ACT/STT scale tables).  PSUM start flag zeroes the whole bank, so only the
first matmul per bank sets start=True.
"""

import math

import numpy as np

_IMG = 512
_B = 2
_F = 33
_R = 51
_TH = 0.001
_NCORES = 8
_NS1 = 11   # blur slots per core
_ND = 10    # dog slots per core

# per-slot radius: slot t's worst filter over cores is q=3's f = min(23+t, 32)
def _slot_radii(sigmas):
    rads = [int(5.0 * float(s) + 0.5) for s in sigmas]
    return [rads[min(23 + t, _F - 1)] for t in range(_NS1)]


def _windows(Rt):
    # (jlo, width) per kt; coverage of band [128kt-R, 128kt+128+R) clipped
    return [
        (0, 128 + Rt),
        (128 - Rt, 128 + 2 * Rt),
        (256 - Rt, 128 + 2 * Rt),
        (384 - Rt, 128 + Rt),
    ]


_cache = {}


def _thn():
    return float(np.nextafter(np.float32(_TH), np.float32(np.inf)))


def _build_host_data(kernels, sigmas):
    fp16 = np.float16
    kernels = np.asarray(kernels, dtype=np.float32)
    sigmas = np.asarray(sigmas, dtype=np.float32)
    F = kernels.shape[0]
    assert F == _F

    # exact 1D taps: kernel = outer(t, t) with t = row / sqrt(center)
    A32 = np.zeros((F, _IMG, _IMG), dtype=np.float32)
    idx = np.arange(_IMG)
    for f in range(F):
        k2 = kernels[f].astype(np.float64)
        taps = k2[_R, : 2 * _R + 1] / math.sqrt(k2[_R, _R])
        A = np.zeros((_IMG, _IMG), dtype=np.float64)
        for d in range(-_R, _R + 1):
            v = taps[_R + d]
            src = idx[max(0, -d): _IMG - max(0, d)]
            A[src, src + d] = v
        A32[f] = A.astype(np.float32)

    radii = _slot_radii(sigmas)

    def pair(x):
        h = x.astype(fp16)
        l = (x - h.astype(np.float32)).astype(fp16)
        return np.ascontiguousarray(h), np.ascontiguousarray(l)

    # per-quarter banded window tensors [NS1, 128, 1024], fp16 pair, x 2^15
    aw_by_q = {}
    for qq in range(4):
        J0 = 8 * qq - 1
        gs = [min(max(J0 + t, 0), _F - 1) for t in range(_NS1)]
        aw = np.zeros((_NS1, 128, 1024), dtype=np.float32)
        for t in range(_NS1):
            f = gs[t]
            for kt, (jlo, w) in enumerate(_windows(radii[t])):
                rows = A32[f][128 * kt: 128 * kt + 128]
                aw[t, :, 256 * kt: 256 * kt + w] = rows[:, jlo: jlo + w]
        aw_by_q[qq] = pair(aw * np.float32(2.0 ** 15))

    return aw_by_q, sigmas, radii, fp16


def _build_program(radii):
    import concourse.bass as bass  # noqa: F401
    import concourse.mybir as mybir
    import concourse.tile as tile
    from concourse import bacc

    fp32 = mybir.dt.float32
    fp16 = mybir.dt.float16
    u8 = mybir.dt.uint8
    Alu = mybir.AluOpType
    Act = mybir.ActivationFunctionType

    nc = bacc.Bacc("TRN2", target_bir_lowering=False)

    Th_d = nc.dram_tensor("timgh", [_IMG, _IMG], fp16, kind="ExternalInput")
    Tl_d = nc.dram_tensor("timgl", [_IMG, _IMG], fp16, kind="ExternalInput")
    awh_d = nc.dram_tensor("awh", [_NS1, 128, 1024], fp16, kind="ExternalInput")
    awl_d = nc.dram_tensor("awl", [_NS1, 128, 1024], fp16, kind="ExternalInput")
    # scale/bias tables: [128, NS1] each
    wsc_d = nc.dram_tensor("wsc", [128, _NS1], fp32, kind="ExternalInput")
    wbi_d = nc.dram_tensor("wbi", [128, _NS1], fp32, kind="ExternalInput")
    dsc_d = nc.dram_tensor("dsc", [128, _NS1], fp32, kind="ExternalInput")
    lm_d = nc.dram_tensor("lm", [8, _IMG, _IMG], fp32, kind="ExternalOutput")
    mk_d = nc.dram_tensor("mask", [8, _IMG, _IMG], u8, kind="ExternalOutput")

    with tile.TileContext(nc) as tc:
        with (
            tc.tile_pool(name="const", bufs=1) as constp,
            tc.tile_pool(name="aw", bufs=2) as awp,
            tc.tile_pool(name="s1", bufs=2) as s1p,
            tc.tile_pool(name="wv", bufs=2) as wvp,
            tc.tile_pool(name="dv", bufs=2) as dvp,
            tc.tile_pool(name="dog", bufs=3) as dogp,
            tc.tile_pool(name="q", bufs=2) as qp,
            tc.tile_pool(name="tri", bufs=1) as trip,
            tc.tile_pool(name="m1", bufs=1) as m1p,
            tc.tile_pool(name="mh", bufs=2) as mhp,
            tc.tile_pool(name="sh", bufs=4) as shp,
            tc.tile_pool(name="hx", bufs=1) as hxp,
            tc.tile_pool(name="lmp", bufs=1) as lmp,
            tc.tile_pool(name="msk", bufs=1) as mskp,
            tc.tile_pool(name="hs", bufs=3, space="DRAM") as hsp,
            tc.tile_pool(name="psA", bufs=4, space="PSUM") as psAp,
            tc.tile_pool(name="psB", bufs=1, space="PSUM") as psBp,
        ):
            Th_sb = constp.tile([128, 4, _IMG], fp16, tag="th")
            nc.sync.dma_start(Th_sb[:], Th_d.rearrange("(t p) y -> p t y", p=128))
            Tl_sb = constp.tile([128, 4, _IMG], fp16, tag="tl")
            nc.sync.dma_start(Tl_sb[:], Tl_d.rearrange("(t p) y -> p t y", p=128))
            wsc_sb = constp.tile([128, _NS1], fp32, tag="wsc")
            nc.sync.dma_start(wsc_sb[:], wsc_d[:])
            wbi_sb = constp.tile([128, _NS1], fp32, tag="wbi")
            nc.sync.dma_start(wbi_sb[:], wbi_d[:])
            dsc_sb = constp.tile([128, _NS1], fp32, tag="dsc")
            nc.sync.dma_start(dsc_sb[:], dsc_d[:])

            AW = {}
            S1 = {}
            WV = {}   # W_t = sigma_t * U_t - thn
            PSB = {}
            DOG = {}
            Q = {}

            def load_aw(t):
                awh = awp.tile([128, 1024], fp16, tag="awh")
                nc.sync.dma_start(awh[:], awh_d[t])
                awl = awp.tile([128, 1024], fp16, tag="awl")
                nc.sync.dma_start(awl[:], awl_d[t])
                AW[t] = (awh, awl)

            def mt_matmuls(t, stat_h, stat_l, out_ap_fn):
                """12 matmuls per mt: 4 kt x 3 hi/lo terms into psum."""
                awh, awl = AW[t]
                wins = _windows(radii[t])
                for mt in range(4):
                    nmm = 0
                    for kt in (0, 3, 1, 2):
                        jlo, w = wins[kt]
                        terms = ((stat_h, awh), (stat_l, awh), (stat_h, awl))
                        for sb, aw in terms:
                            nc.tensor.matmul(
                                out_ap_fn(mt, jlo, w),
                                sb[:, kt, 128 * mt: 128 * mt + 128],
                                aw[:, 256 * kt: 256 * kt + w],
                                start=(nmm == 0),
                                stop=(nmm == 11),
                            )
                            nmm += 1

            def slot(t):
                # pass1: 4 separate 1-bank psum tiles for per-mt eviction
                # pipelining (PE can roll into pass2 while evictions drain)
                psA = [psAp.tile([128, _IMG], fp32, tag="ps", name=f"psA{t}_{m}")
                       for m in range(4)]
                mt_matmuls(t, Th_sb, Tl_sb,
                           lambda mt, jlo, w: psA[mt][:, jlo: jlo + w])
                s1h = s1p.tile([128, 4, _IMG], fp16, tag="s1h")
                s1l = s1p.tile([128, 4, _IMG], fp16, tag="s1l")
                for mt in range(4):
                    nc.scalar.activation(s1h[:, mt, :], psA[mt][:], Act.Copy)
                    nc.vector.tensor_tensor(
                        s1l[:, mt, :], psA[mt][:], s1h[:, mt, :], Alu.subtract)
                # pass2: one 4-bank psum tile (consumers are not PE-critical)
                psB = psBp.tile([128, 4, _IMG], fp32, tag="psb")
                mt_matmuls(t, s1h, s1l,
                           lambda mt, jlo, w: psB[:, mt, jlo: jlo + w])
                S1[t] = (s1h, s1l)
                if t - 1 in S1:
                    del S1[t - 1]
                # D_t = psB_t * dsc[t] FIRST (it gates dog_{t-1} on the
                # critical path); W_t feeds only the NEXT slot's dog.
                if t >= 1:
                    dsb = dvp.tile([128, 4, _IMG], fp32, tag="dv")
                    nc.scalar.activation(
                        dsb[:], psB[:], Act.Identity,
                        scale=dsc_sb[:, t: t + 1])
                    d = dogp.tile([128, 4, _IMG], fp32, tag="dog")
                    nc.gpsimd.tensor_tensor(
                        d[:], dsb[:], WV[t - 1][:], Alu.add)
                    DOG[t - 1] = d
                    del WV[t - 1]
                # W_t eviction (scalar): W = psB*wsc[t] + wbi[t]
                wv = wvp.tile([128, 4, _IMG], fp32, tag="wv")
                nc.scalar.activation(
                    wv[:], psB[:], Act.Identity,
                    scale=wsc_sb[:, t: t + 1], bias=wbi_sb[:, t: t + 1])
                WV[t] = wv

            def make_q(x):
                # q_x = max(dog_x, dog_{x+1})   (max only exists on DVE)
                qt = qp.tile([128, 4, _IMG], fp32, tag="q")
                nc.vector.tensor_tensor(qt[:], DOG[x][:], DOG[x + 1][:], Alu.max)
                Q[x] = qt

            SH = {}

            def pool_front(o):
                # output plane o (0..7): tri over dogs o, o+1, o+2; then the
                # free-dim (H) 3-max and the W round-trip DMAs.  The round
                # trip completes during the NEXT slot; pool_back picks it up.
                tri = trip.tile([128, 4, _IMG], fp32, tag="tri")
                nc.vector.tensor_tensor(
                    tri[:], Q[o][:], DOG[o + 2][:], Alu.max)
                if o - 1 in Q:
                    del Q[o - 1]

                # H (y, free dim) 3-max; edge columns via scalar copies
                m1 = m1p.tile([128, 4, _IMG], fp32, tag="m1")
                nc.vector.tensor_tensor(
                    m1[:, :, 0:511], tri[:, :, 0:511], tri[:, :, 1:512],
                    Alu.max)
                nc.scalar.activation(
                    m1[:, :, 511:512], tri[:, :, 511:512], Act.Copy)
                mh = mhp.tile([128, 4, _IMG], fp32, tag="mh")
                nc.vector.tensor_tensor(
                    mh[:, :, 1:512], m1[:, :, 0:511], m1[:, :, 1:512], Alu.max)
                nc.scalar.activation(mh[:, :, 0:1], m1[:, :, 0:1], Act.Copy)

                # W (x = partition dim) shifts via DRAM round trip, edge clamp
                hs = hsp.tile([514, _IMG], fp32, tag="hs")
                nc.sync.dma_start(
                    hs[1:513].rearrange("(m p) y -> p m y", p=128), mh[:])
                nc.scalar.dma_start(hs[0:1], mh[0:1, 0:1, :])
                nc.scalar.dma_start(hs[513:514], mh[127:128, 3:4, :])
                shA = shp.tile([128, 4, _IMG], fp32, tag="sh")
                nc.sync.dma_start(
                    shA[:], hs[0:512].rearrange("(m p) y -> p m y", p=128))
                shB = shp.tile([128, 4, _IMG], fp32, tag="sh")
                nc.sync.dma_start(
                    shB[:], hs[2:514].rearrange("(m p) y -> p m y", p=128))
                SH[o] = (mh, shA, shB)

            def pool_back(o):
                mh, shA, shB = SH.pop(o)
                hA = hxp.tile([128, 4, _IMG], fp32, tag="hx")
                nc.vector.tensor_tensor(hA[:], mh[:], shA[:], Alu.max)
                lmT = lmp.tile([128, 4, _IMG], fp32, tag="lm")
                nc.vector.tensor_tensor(lmT[:], hA[:], shB[:], Alu.max)
                nc.sync.dma_start(
                    lm_d[o].rearrange("(m p) y -> p m y", p=128), lmT[:])

                # mask = ((lm' max 0) == dog'_{o+1})   (one fused STT)
                mk = mskp.tile([128, 4, _IMG], u8, tag="msk")
                nc.vector.scalar_tensor_tensor(
                    mk[:], lmT[:], 0.0, DOG[o + 1][:], Alu.max, Alu.is_equal)
                nc.sync.dma_start(
                    mk_d[o].rearrange("(m p) y -> p m y", p=128), mk[:])
                if o in DOG:
                    del DOG[o]

            load_aw(0)
            for t in range(_NS1):
                if t + 1 < _NS1:
                    load_aw(t + 1)
                slot(t)
                # pool_back first: its inputs (last slot's round trip) are
                # already in flight, so the vector engine has ready work
                # while this slot's dog chain (scalar D -> gpsimd add)
                # resolves; q/front depend on the fresh dog.
                if t >= 4:
                    pool_back(t - 4)
                if 2 <= t <= 9:
                    make_q(t - 2)
                if t >= 3:
                    pool_front(t - 3)
            pool_back(7)

    nc.compile()
    return nc


def kernel(input, kernels, sigmas):
    from concourse.bass_utils import run_bass_kernel_spmd

    input = np.asarray(input, dtype=np.float32)
    aw_by_q, sig, radii, bf16 = _cache.setdefault(
        "host", _build_host_data(kernels, sigmas))

    if "prog" not in _cache:
        _cache["prog"] = _build_program(radii)
    nc = _cache["prog"]

    thn = _thn()
    in_maps = []
    for c in range(_NCORES):
        b, qq = c // 4, c % 4
        J0 = 8 * qq - 1

        T = np.ascontiguousarray(np.transpose(input[b]))  # [x, y]
        Th = T.astype(bf16)
        Tl = (T - Th.astype(np.float32)).astype(bf16)

        wsc = np.zeros((128, _NS1), dtype=np.float32)
        wbi = np.zeros((128, _NS1), dtype=np.float32)
        dsc = np.zeros((128, _NS1), dtype=np.float32)
        for t in range(_NS1):
            j = J0 + t
            if 0 <= j < _F - 1:
                wsc[:, t] = sig[j] * np.float32(2.0 ** -30)
                wbi[:, t] = -thn
            else:
                wsc[:, t] = 0.0
                wbi[:, t] = -1e38
            jd = J0 + t - 1
            if 0 <= jd < _F - 1:
                dsc[:, t] = -sig[jd] * np.float32(2.0 ** -30)
            else:
                dsc[:, t] = 0.0
        awh, awl = aw_by_q[qq]
        in_maps.append({
            "timgh": Th, "timgl": Tl,
            "awh": awh, "awl": awl,
            "wsc": wsc, "wbi": wbi, "dsc": dsc,
        })

    res = run_bass_kernel_spmd(
        nc, in_maps, core_ids=list(range(_NCORES)),
        trace=_cache.get("trace", False),
        tmpdir=_cache.get("tmpdir"),
    )
    _cache["last_res"] = res

    lm_full = np.empty((_B, _F - 1, _IMG, _IMG), dtype=np.float32)
    mk_full = np.empty((_B, _F - 1, _IMG, _IMG), dtype=bool)
    for c in range(_NCORES):
        b, qq = c // 4, c % 4
        lm_c = res.results[c]["lm"]      # [8, x, y]
        mk_c = res.results[c]["mask"]
        lm_full[b, 8 * qq: 8 * qq + 8] = (
            np.transpose(lm_c, (0, 2, 1)) + np.float32(thn))
        mk_full[b, 8 * qq: 8 * qq + 8] = np.transpose(mk_c, (0, 2, 1)) != 0
    return mk_full, lm_full


# revision 36
# speedup vs baseline: 1.1683x; 1.0176x over previous
"""DifferenceOfGaussiansFFT on 8 Trainium2 NeuronCores — v3.

Sharding: core c -> (batch b = c//4, quarter q = c%4).  Each core computes
dog planes [8q, 8q+8) for its batch plus one halo plane each side
(recompute, no collectives): 11 blur slots, 10 dogs, 8 pools per core.

Math per core (all layouts TRANSPOSED: partition = x = W, free = y = H):
    pass1: S1[y, x'] = sum_x T[x, y] A[x, x']     (stationary image blocks,
           moving banded A windows; per-slot window width 128+2R_t)
    pass2: U^T[x', y'] = sum_y S1[y, x'] A[y, y'] (stationary S1 blocks,
           SAME moving A windows — A is symmetric)
    W_t   = sigma_t*U_t - thn                (ACT eviction, scale+bias)
    dog_{t-1} = psB_t*(-sigma_{t-1}*2^-30) + W_{t-1}   (one STT from PSUM)
    lm'   = maxpool3d(dog')  (F via q/tri, H free-dim shifts, W via DRAM
            round trip for partition shifts)
    mask  = ((lm' max 0) is_equal dog')      (one fused STT, u8)
    host adds thn back and transposes [x,y] -> [y,x].

Matmuls are bf16-pair fp16 hi/lo, 3 terms (Ah*Bh + Al*Bh + Ah*Bl),
weights scaled 2^15 per pass (U comes out 2^30-scaled; compensated in the
ACT/STT scale tables).  PSUM start flag zeroes the whole bank, so only the
first matmul per bank sets start=True.
"""

import math

import numpy as np

_IMG = 512
_B = 2
_F = 33
_R = 51
_TH = 0.001
_NCORES = 8
_NS1 = 11   # blur slots per core
_ND = 10    # dog slots per core

# per-slot radius: slot t's worst filter over cores is q=3's f = min(23+t, 32)
def _slot_radii(sigmas):
    rads = [int(5.0 * float(s) + 0.5) for s in sigmas]
    return [rads[min(23 + t, _F - 1)] for t in range(_NS1)]


def _windows(Rt):
    # (jlo, width) per kt; coverage of band [128kt-R, 128kt+128+R) clipped
    return [
        (0, 128 + Rt),
        (128 - Rt, 128 + 2 * Rt),
        (256 - Rt, 128 + 2 * Rt),
        (384 - Rt, 128 + Rt),
    ]


_cache = {}


def _thn():
    return float(np.nextafter(np.float32(_TH), np.float32(np.inf)))


def _build_host_data(kernels, sigmas):
    fp16 = np.float16
    kernels = np.asarray(kernels, dtype=np.float32)
    sigmas = np.asarray(sigmas, dtype=np.float32)
    F = kernels.shape[0]
    assert F == _F

    # exact 1D taps: kernel = outer(t, t) with t = row / sqrt(center)
    A32 = np.zeros((F, _IMG, _IMG), dtype=np.float32)
    idx = np.arange(_IMG)
    for f in range(F):
        k2 = kernels[f].astype(np.float64)
        taps = k2[_R, : 2 * _R + 1] / math.sqrt(k2[_R, _R])
        A = np.zeros((_IMG, _IMG), dtype=np.float64)
        for d in range(-_R, _R + 1):
            v = taps[_R + d]
            src = idx[max(0, -d): _IMG - max(0, d)]
            A[src, src + d] = v
        A32[f] = A.astype(np.float32)

    radii = _slot_radii(sigmas)

    def pair(x):
        h = x.astype(fp16)
        l = (x - h.astype(np.float32)).astype(fp16)
        return np.ascontiguousarray(h), np.ascontiguousarray(l)

    # per-quarter banded window tensors [NS1, 128, 1024], fp16 pair, x 2^15
    aw_by_q = {}
    for qq in range(4):
        J0 = 8 * qq - 1
        gs = [min(max(J0 + t, 0), _F - 1) for t in range(_NS1)]
        aw = np.zeros((_NS1, 128, 1024), dtype=np.float32)
        for t in range(_NS1):
            f = gs[t]
            for kt, (jlo, w) in enumerate(_windows(radii[t])):
                rows = A32[f][128 * kt: 128 * kt + 128]
                aw[t, :, 256 * kt: 256 * kt + w] = rows[:, jlo: jlo + w]
        aw_by_q[qq] = pair(aw * np.float32(2.0 ** 15))

    return aw_by_q, sigmas, radii, fp16


def _build_program(radii):
    import concourse.bass as bass  # noqa: F401
    import concourse.mybir as mybir
    import concourse.tile as tile
    from concourse import bacc

    fp32 = mybir.dt.float32
    fp16 = mybir.dt.float16
    u8 = mybir.dt.uint8
    Alu = mybir.AluOpType
    Act = mybir.ActivationFunctionType

    nc = bacc.Bacc("TRN2", target_bir_lowering=False)

    Th_d = nc.dram_tensor("timgh", [_IMG, _IMG], fp16, kind="ExternalInput")
    Tl_d = nc.dram_tensor("timgl", [_IMG, _IMG], fp16, kind="ExternalInput")
    awh_d = nc.dram_tensor("awh", [_NS1, 128, 1024], fp16, kind="ExternalInput")
    awl_d = nc.dram_tensor("awl", [_NS1, 128, 1024], fp16, kind="ExternalInput")
    # scale/bias tables: [128, NS1] each
    wsc_d = nc.dram_tensor("wsc", [128, _NS1], fp32, kind="ExternalInput")
    wbi_d = nc.dram_tensor("wbi", [128, _NS1], fp32, kind="ExternalInput")
    dsc_d = nc.dram_tensor("dsc", [128, _NS1], fp32, kind="ExternalInput")
    lm_d = nc.dram_tensor("lm", [8, _IMG, _IMG], fp32, kind="ExternalOutput")
    mk_d = nc.dram_tensor("mask", [8, _IMG, _IMG], u8, kind="ExternalOutput")

    with tile.TileContext(nc) as tc:
        with (
            tc.tile_pool(name="const", bufs=1) as constp,
            tc.tile_pool(name="aw", bufs=2) as awp,
            tc.tile_pool(name="s1", bufs=2) as s1p,
            tc.tile_pool(name="wv", bufs=2) as wvp,
            tc.tile_pool(name="dv", bufs=2) as dvp,
            tc.tile_pool(name="dog", bufs=3) as dogp,
            tc.tile_pool(name="q", bufs=2) as qp,
            tc.tile_pool(name="tri", bufs=1) as trip,
            tc.tile_pool(name="m1", bufs=1) as m1p,
            tc.tile_pool(name="mh", bufs=2) as mhp,
            tc.tile_pool(name="sh", bufs=4) as shp,
            tc.tile_pool(name="hx", bufs=1) as hxp,
            tc.tile_pool(name="lmp", bufs=1) as lmp,
            tc.tile_pool(name="msk", bufs=1) as mskp,
            tc.tile_pool(name="hs", bufs=3, space="DRAM") as hsp,
            tc.tile_pool(name="psA", bufs=4, space="PSUM") as psAp,
            tc.tile_pool(name="psB", bufs=1, space="PSUM") as psBp,
        ):
            # image loads split per kt-block so pass1's first matmuls can
            # start as soon as their stationary block lands
            Th_sb = constp.tile([128, 4, _IMG], fp16, tag="th")
            Tl_sb = constp.tile([128, 4, _IMG], fp16, tag="tl")
            for k in (0, 3, 1, 2):
                nc.sync.dma_start(
                    Th_sb[:, k, :],
                    Th_d.rearrange("(t p) y -> p t y", p=128)[:, k, :])
                nc.sync.dma_start(
                    Tl_sb[:, k, :],
                    Tl_d.rearrange("(t p) y -> p t y", p=128)[:, k, :])
            wsc_sb = constp.tile([128, _NS1], fp32, tag="wsc")
            nc.sync.dma_start(wsc_sb[:], wsc_d[:])
            wbi_sb = constp.tile([128, _NS1], fp32, tag="wbi")
            nc.sync.dma_start(wbi_sb[:], wbi_d[:])
            dsc_sb = constp.tile([128, _NS1], fp32, tag="dsc")
            nc.sync.dma_start(dsc_sb[:], dsc_d[:])

            AW = {}
            S1 = {}
            WV = {}   # W_t = sigma_t * U_t - thn
            PSB = {}
            DOG = {}
            Q = {}

            def load_aw(t):
                awh = awp.tile([128, 1024], fp16, tag="awh")
                nc.sync.dma_start(awh[:], awh_d[t])
                awl = awp.tile([128, 1024], fp16, tag="awl")
                nc.sync.dma_start(awl[:], awl_d[t])
                AW[t] = (awh, awl)

            def mt_matmuls(t, stat_h, stat_l, out_ap_fn):
                """12 matmuls per mt: 4 kt x 3 hi/lo terms into psum."""
                awh, awl = AW[t]
                wins = _windows(radii[t])
                for mt in range(4):
                    nmm = 0
                    for kt in (0, 3, 1, 2):
                        jlo, w = wins[kt]
                        terms = ((stat_h, awh), (stat_l, awh), (stat_h, awl))
                        for sb, aw in terms:
                            nc.tensor.matmul(
                                out_ap_fn(mt, jlo, w),
                                sb[:, kt, 128 * mt: 128 * mt + 128],
                                aw[:, 256 * kt: 256 * kt + w],
                                start=(nmm == 0),
                                stop=(nmm == 11),
                            )
                            nmm += 1

            def slot(t):
                # pass1: 4 separate 1-bank psum tiles for per-mt eviction
                # pipelining (PE can roll into pass2 while evictions drain)
                psA = [psAp.tile([128, _IMG], fp32, tag="ps", name=f"psA{t}_{m}")
                       for m in range(4)]
                mt_matmuls(t, Th_sb, Tl_sb,
                           lambda mt, jlo, w: psA[mt][:, jlo: jlo + w])
                s1h = s1p.tile([128, 4, _IMG], fp16, tag="s1h")
                s1l = s1p.tile([128, 4, _IMG], fp16, tag="s1l")
                for mt in range(4):
                    nc.scalar.activation(s1h[:, mt, :], psA[mt][:], Act.Copy)
                    nc.vector.tensor_tensor(
                        s1l[:, mt, :], psA[mt][:], s1h[:, mt, :], Alu.subtract)
                # pass2: one 4-bank psum tile (consumers are not PE-critical)
                psB = psBp.tile([128, 4, _IMG], fp32, tag="psb")
                mt_matmuls(t, s1h, s1l,
                           lambda mt, jlo, w: psB[:, mt, jlo: jlo + w])
                S1[t] = (s1h, s1l)
                if t - 1 in S1:
                    del S1[t - 1]
                # D_t = psB_t * dsc[t] FIRST (it gates dog_{t-1} on the
                # critical path); W_t feeds only the NEXT slot's dog.
                # Processed in 2 halves so the downstream q can start on
                # the first half while the second drains.
                if t >= 1:
                    dsb = dvp.tile([128, 4, _IMG], fp32, tag="dv")
                    d = dogp.tile([128, 4, _IMG], fp32, tag="dog")
                    for h in range(2):
                        sl = slice(2 * h, 2 * h + 2)
                        nc.scalar.activation(
                            dsb[:, sl, :], psB[:, sl, :], Act.Identity,
                            scale=dsc_sb[:, t: t + 1])
                        nc.gpsimd.tensor_tensor(
                            d[:, sl, :], dsb[:, sl, :], WV[t - 1][:, sl, :],
                            Alu.add)
                    DOG[t - 1] = d
                    del WV[t - 1]
                # W_t eviction (scalar): W = psB*wsc[t] + wbi[t]
                wv = wvp.tile([128, 4, _IMG], fp32, tag="wv")
                nc.scalar.activation(
                    wv[:], psB[:], Act.Identity,
                    scale=wsc_sb[:, t: t + 1], bias=wbi_sb[:, t: t + 1])
                WV[t] = wv

            def make_q(x):
                # q_x = max(dog_x, dog_{x+1})   (max only exists on DVE);
                # halved to chase the dog halves down the pipeline
                qt = qp.tile([128, 4, _IMG], fp32, tag="q")
                for h in range(2):
                    sl = slice(2 * h, 2 * h + 2)
                    nc.vector.tensor_tensor(
                        qt[:, sl, :], DOG[x][:, sl, :], DOG[x + 1][:, sl, :],
                        Alu.max)
                Q[x] = qt

            SH = {}

            def pool_front(o):
                # output plane o (0..7): tri over dogs o, o+1, o+2; then the
                # free-dim (H) 3-max and the W round-trip DMAs.  The round
                # trip completes during the NEXT slot; pool_back picks it up.
                tri = trip.tile([128, 4, _IMG], fp32, tag="tri")
                nc.vector.tensor_tensor(
                    tri[:], Q[o][:], DOG[o + 2][:], Alu.max)
                if o - 1 in Q:
                    del Q[o - 1]

                # H (y, free dim) 3-max; edge columns via scalar copies
                m1 = m1p.tile([128, 4, _IMG], fp32, tag="m1")
                nc.vector.tensor_tensor(
                    m1[:, :, 0:511], tri[:, :, 0:511], tri[:, :, 1:512],
                    Alu.max)
                nc.scalar.activation(
                    m1[:, :, 511:512], tri[:, :, 511:512], Act.Copy)
                mh = mhp.tile([128, 4, _IMG], fp32, tag="mh")
                nc.vector.tensor_tensor(
                    mh[:, :, 1:512], m1[:, :, 0:511], m1[:, :, 1:512], Alu.max)
                nc.scalar.activation(mh[:, :, 0:1], m1[:, :, 0:1], Act.Copy)

                # W (x = partition dim) shifts via DRAM round trip, edge clamp
                hs = hsp.tile([514, _IMG], fp32, tag="hs")
                nc.sync.dma_start(
                    hs[1:513].rearrange("(m p) y -> p m y", p=128), mh[:])
                nc.scalar.dma_start(hs[0:1], mh[0:1, 0:1, :])
                nc.scalar.dma_start(hs[513:514], mh[127:128, 3:4, :])
                shA = shp.tile([128, 4, _IMG], fp32, tag="sh")
                nc.sync.dma_start(
                    shA[:], hs[0:512].rearrange("(m p) y -> p m y", p=128))
                shB = shp.tile([128, 4, _IMG], fp32, tag="sh")
                nc.sync.dma_start(
                    shB[:], hs[2:514].rearrange("(m p) y -> p m y", p=128))
                SH[o] = (mh, shA, shB)

            def pool_back(o):
                mh, shA, shB = SH.pop(o)
                hA = hxp.tile([128, 4, _IMG], fp32, tag="hx")
                nc.vector.tensor_tensor(hA[:], mh[:], shA[:], Alu.max)
                lmT = lmp.tile([128, 4, _IMG], fp32, tag="lm")
                nc.vector.tensor_tensor(lmT[:], hA[:], shB[:], Alu.max)
                nc.sync.dma_start(
                    lm_d[o].rearrange("(m p) y -> p m y", p=128), lmT[:])

                # mask = ((lm' max 0) == dog'_{o+1})   (one fused STT)
                mk = mskp.tile([128, 4, _IMG], u8, tag="msk")
                nc.vector.scalar_tensor_tensor(
                    mk[:], lmT[:], 0.0, DOG[o + 1][:], Alu.max, Alu.is_equal)
                nc.sync.dma_start(
                    mk_d[o].rearrange("(m p) y -> p m y", p=128), mk[:])
                if o in DOG:
                    del DOG[o]

            load_aw(0)
            for t in range(_NS1):
                if t + 1 < _NS1:
                    load_aw(t + 1)
                slot(t)
                # pool_back first: its inputs (last slot's round trip) are
                # already in flight, so the vector engine has ready work
                # while this slot's dog chain (scalar D -> gpsimd add)
                # resolves; q/front depend on the fresh dog.
                if t >= 4:
                    pool_back(t - 4)
                if 2 <= t <= 9:
                    make_q(t - 2)
                if t >= 3:
                    pool_front(t - 3)
            pool_back(7)

    nc.compile()
    return nc


def kernel(input, kernels, sigmas):
    from concourse.bass_utils import run_bass_kernel_spmd

    input = np.asarray(input, dtype=np.float32)
    aw_by_q, sig, radii, bf16 = _cache.setdefault(
        "host", _build_host_data(kernels, sigmas))

    if "prog" not in _cache:
        _cache["prog"] = _build_program(radii)
    nc = _cache["prog"]

    thn = _thn()
    in_maps = []
    for c in range(_NCORES):
        b, qq = c // 4, c % 4
        J0 = 8 * qq - 1

        T = np.ascontiguousarray(np.transpose(input[b]))  # [x, y]
        Th = T.astype(bf16)
        Tl = (T - Th.astype(np.float32)).astype(bf16)

        wsc = np.zeros((128, _NS1), dtype=np.float32)
        wbi = np.zeros((128, _NS1), dtype=np.float32)
        dsc = np.zeros((128, _NS1), dtype=np.float32)
        for t in range(_NS1):
            j = J0 + t
            if 0 <= j < _F - 1:
                wsc[:, t] = sig[j] * np.float32(2.0 ** -30)
                wbi[:, t] = -thn
            else:
                wsc[:, t] = 0.0
                wbi[:, t] = -1e38
            jd = J0 + t - 1
            if 0 <= jd < _F - 1:
                dsc[:, t] = -sig[jd] * np.float32(2.0 ** -30)
            else:
                dsc[:, t] = 0.0
        awh, awl = aw_by_q[qq]
        in_maps.append({
            "timgh": Th, "timgl": Tl,
            "awh": awh, "awl": awl,
            "wsc": wsc, "wbi": wbi, "dsc": dsc,
        })

    res = run_bass_kernel_spmd(
        nc, in_maps, core_ids=list(range(_NCORES)),
        trace=_cache.get("trace", False),
        tmpdir=_cache.get("tmpdir"),
    )
    _cache["last_res"] = res

    lm_full = np.empty((_B, _F - 1, _IMG, _IMG), dtype=np.float32)
    mk_full = np.empty((_B, _F - 1, _IMG, _IMG), dtype=bool)
    for c in range(_NCORES):
        b, qq = c // 4, c % 4
        lm_c = res.results[c]["lm"]      # [8, x, y]
        mk_c = res.results[c]["mask"]
        lm_full[b, 8 * qq: 8 * qq + 8] = (
            np.transpose(lm_c, (0, 2, 1)) + np.float32(thn))
        mk_full[b, 8 * qq: 8 * qq + 8] = np.transpose(mk_c, (0, 2, 1)) != 0
    return mk_full, lm_full
